# revision 56
# baseline (speedup 1.0000x reference)
"""Trainium2 Bass kernel for nn_Attention_30554397344218.

Multi-head attention (B=8, S=1040, D=1024, H=16, hd=64) with 2D vision RoPE
on the 1024 grid tokens after a 16-token puzzle prefix.

Sharding: pure data-parallel - one batch element per NeuronCore (8 cores,
no collectives); weights broadcast; host gathers the 8 outputs.

v3 design, 318.5us -> ~308us. Trace shows the PE at ~93% union occupancy
end-to-end - the kernel is Tensor-bound, NOT phase-2-ACT-bound as v2
believed (ACT exp is 63% of its window) - so v3 cuts PE columns and PE
stalls (all-bf16 matmuls; fp8 rejected: rel err 2.4-5.2e-2 vs the 2e-2
gate because softmax does not damp relative error):
  - RoPE swap WITHOUT the PE permutation matmul (-16.6k cols = -7us): hd
    components stored [r1,c1,r2,c2] per head (weights+tables permuted
    host-side, HD_PERM), so rotate-half partner = partition p^32 -> 4
    contiguous-range SBUF->SBUF DMAs per chunk on the idle sync queue.
    Scores/att@v are invariant to the permutation (q,k share it).
  - exp split ACT/DVE: key tiles j in {6,7} + their query tails exp'd on
    the Vector engine via one-op Schraudolph-to-bf16-bits (tensor_scalar
    f32->i16, A/65536, B/65536; i16 rn == bf16 bits; att@v reads
    .bitcast(bf16)). -37us ACT keeps the st-slot recycle off the exp
    critical path. rel err 7.1e-3 -> 1.04e-2 (gate 2e-2; exp_acc.py
    models tile-set choices). Head 15's {6,7} go on ACT instead (DVE is
    the drain's critical path), writing through pti[0].bitcast - ~2us.
    j-set {5,6} tried: breaks the weave, HAM oscillates, +70us. NJD=3
    ({5,6,7}) tried: +4us (DVE congestion).
  - mm1 i-outer (psum chunk per ICH slice); v j-tiles 0-2 woven mid
    phase-1 (k3-k5 windows), NOT at the end: the phase-1->2 psum pool
    turnover barrier waits on the last phase-1 psum consumer (-2us).
  - drain: it3-8 out-proj on per-ci [128,512] po tiles (3-ring) instead
    of the 2-slot [128,1024] st ring; ysb ring 4 (the DMA latency chained
    copy->dma->copy at ring 2); output DMAs only on sync/gpsimd - a
    nc.scalar.dma_start issue between ACT ysb copies stalls the drain.
  - inputs host-packed PIECE-MAJOR and contiguous on BOTH dma sides
    (xt by ICH column block as [p,k,c]; wq/wk by 128-col chunk and wv by
    512-half as [c][p][k][m], with the sbuf weight tiles chunk-major to
    match): mm1(q0) needs only the first ~1.3MB off the wire and starts
    ~15us instead of ~20us. Strided sbuf dests fragment DMA into 256B
    packets and tank bandwidth - both sides must be contiguous per
    partition. Warmup 12 matmuls covers the (shorter) dma wait.
  - kept from v2: transposed q/k layout (2 heads/chunk, k duplicated
    half-zeroed for K=128 scores), 3-head j8 group packing at offsets
    0/32/64, st 2x2 + tl 1 + po 3 psum banks, bf16 selector norm
    matmuls, otc reuses qr slots, it0/1/2 accumulate chunks 0-6 early.
  - att@v weave slots: part1 at j in {0,2,4}, part2 at {1,3,5} (one slot
    earlier than v2's {1,4,6}/{2,5,7}: -4us of st-slot waits). The ACT
    tails exp (j0-5 columns of tl) is emitted right after scores j5, not
    after j7: ACT finishes each head ~1us earlier, which unblocks the
    next head's exps in the st-slot chain. Group 5 (head 15's j8 scores
    + exp) hoisted into h14's window, off the drain gate.
  - the 3 hw DMA queues share ~300 GB/s AGGREGATE (not 105 each): early
    streaming of late-needed tensors steals bandwidth from the critical
    first pieces. wk/wv ride as single large descriptors behind wq on
    scalar (each dma_start issue also costs ~0.65us of engine queue).
    reciprocal_approx_* requires fp32 output (bit-trick seed) - a direct
    bf16 reciprocal is not possible.
  - measured (fast state): ~303-309us (median ~307); last mm ends
    ~300us, PE union ~282us (~94%), PE gaps ~15us, exps end ~263us. Device is bimodal run-to-run: ~312us fast
    vs ~387us degraded (both engines ~20% slower; NEURON_RT_RESET_CORES=1
    + rerun recovers).
  - moving att@v psum evacuations DVE->ACT tried: +17us (next head's
    exps queue behind them in ACT's FIFO - head-of-line).
  - LDWEIGHTS elision is not possible from bass (walrus emits one per
    matmul; InstMatmult.ldweights=False is ignored by the lowering -
    verified on hw); ~30ns/mm pacing above the 213ns streaming floor.
"""

import numpy as np
import ml_dtypes

B, S, D, H, HD = 8, 1040, 1024, 16, 64
PFX = 16
GRID = 32
NCHUNK = 8
NJT = 9
TAIL = S - 8 * 128  # 16
ICH3 = [(0, 512), (512, 512), (1024, 16)]
ICH2 = [(0, 512), (512, 512)]
NPT = 3  # pt ring buffers (lag-2 att@v pipeline)
BF16 = ml_dtypes.bfloat16

_compiled = None


# hd components are stored per head as [r1, c1, r2, c2] (16 partitions each)
# instead of the reference's [r1, r2, c1, c2]: the rope rotate-half partner is
# then partition p ^ 32, so the "swap" is 4 contiguous-range SBUF->SBUF DMA
# copies instead of a PE permutation matmul. Weights columns and the cos/sin
# tables are permuted host-side to match; scores/attention are invariant to
# any per-head hd permutation applied to both q and k.
HD_PERM = (
    list(range(0, 16)) + list(range(32, 48))
    + list(range(16, 32)) + list(range(48, 64))
)


def _rope_tables():
    half, quarter = HD // 2, HD // 4
    frac = 2.0 * np.arange(quarter, dtype=np.float64) / half
    ts = 10000.0 ** frac
    row = np.arange(GRID, dtype=np.float64)[:, None] / ts[None, :]
    row_ang = np.broadcast_to(row[:, None, :], (GRID, GRID, quarter)).reshape(
        GRID * GRID, quarter
    )
    col_ang = np.broadcast_to(row[None, :, :], (GRID, GRID, quarter)).reshape(
        GRID * GRID, quarter
    )
    cos64 = np.concatenate(
        [np.cos(row_ang).T, np.cos(row_ang).T, np.cos(col_ang).T, np.cos(col_ang).T],
        axis=0,
    )
    s64 = np.concatenate(
        [-np.sin(row_ang).T, np.sin(row_ang).T, -np.sin(col_ang).T, np.sin(col_ang).T],
        axis=0,
    )
    cosf = np.ones((HD, S), np.float64)
    sf = np.zeros((HD, S), np.float64)
    cosf[:, PFX:] = cos64
    sf[:, PFX:] = s64
    cosf, sf = cosf[HD_PERM], sf[HD_PERM]
    cos2 = np.concatenate([cosf, cosf], axis=0).astype(BF16)
    s2 = np.concatenate([sf, sf], axis=0).astype(BF16)
    return cos2, s2


def _build_body(nc, tc, tile, mybir, aps):
    from contextlib import ExitStack

    from concourse.alu_op_type import AluOpType

    bf = mybir.dt.bfloat16
    f32 = mybir.dt.float32
    i16 = mybir.dt.int16
    Exp = mybir.ActivationFunctionType.Exp
    # One-op DVE Schraudolph exp, bf16-bits variant (folds the 1/sqrt(HD)
    # score scale and the >>16): i16(rn(A*x + B)) bits == bf16(exp(x/8)),
    # rms err ~1.9%. Key tiles j in {6,7} (2/8 of keys) run on DVE, relieving
    # the ACT engine which otherwise paces phase 2; end-to-end rel err
    # modeled at ~1.05e-2 vs the 2e-2 gate (exp_acc.py).
    SCHR_A = float(0.125 * (1 << 23) / np.log(2.0) / 65536.0)
    SCHR_B = float((127.0 * (1 << 23) - 486411.0) / 65536.0)
    NJD = 2  # j-tiles per head exp'd on DVE
    JSP = 6  # DVE handles j in [JSP, JSP+NJD); ACT the rest

    def ptreg(j):
        # bf16-pt region index for ACT-exp'd j-tiles (j7 packs after j0-4)
        return j if j < JSP else j - NJD
    xT, Wq, Wk, Wv, Wo = aps["xT"], aps["Wq"], aps["Wk"], aps["Wv"], aps["Wo"]
    COS2, S2, SEL2, OUT = aps["COS2"], aps["S2"], aps["SEL2"], aps["out"]

    def rows_of(j):
        return 128 if j < 8 else TAIL

    with ExitStack() as ctx:
        # ---- persistent SBUF pools (live across both phases)
        p_tab = ctx.enter_context(tc.tile_pool(name="tab", bufs=1))
        p_xt = ctx.enter_context(tc.tile_pool(name="xt", bufs=1))
        p_wv = ctx.enter_context(tc.tile_pool(name="wv", bufs=1))
        p_qk = ctx.enter_context(tc.tile_pool(name="qk", bufs=24))
        p_vx = ctx.enter_context(tc.tile_pool(name="vx", bufs=9))

        sel2 = p_tab.tile([2, 128], bf, tag="sel2")
        xt = p_xt.tile([128, 8, S], bf, tag="xt")
        wv_t = p_wv.tile([128, 2, 8, 512], bf, tag="wv")

        qr = [p_qk.tile([128, S], bf, tag="qk", name=f"qr{i}") for i in range(NCHUNK)]
        # krz is 16 columns wider than S: zeroed key-columns that let the
        # group-j8 score matmuls emit 32-tall strips (16 real keys + 16
        # zero rows), so the packed psum tile needs no gap memset
        krz = [
            [p_qk.tile([128, S + 16], bf, tag="qk", name=f"krz{i}_{z}") for z in range(2)]
            for i in range(NCHUNK)
        ]
        vx = [p_vx.tile([128, 1104], bf, tag="vx", name=f"vx{i}") for i in range(NJT)]
        # otc[c] is created lazily at its first att@v write (head 2c+2's
        # weave): pool rotation then hands it qr[c]'s slot, whose last
        # reader (scores of head 2c+1) is already done - a free 16.6KB.
        otc = [None] * NCHUNK

        # ---- input DMAs, priority-ordered and queue-parallel: each DMA
        # queue moves ~105 GB/s, so the tensors needed first (xt, wq) are
        # split across queues in contiguous k-blocks (column splits fragment
        # the dram side into 512B runs and tank DMA bandwidth) in need-order
        # (wq -> wk -> wv).
        # inputs are host-packed PIECE-MAJOR (xt by ICH column block, w by
        # 128-col chunk) so the ~1.3MB that mm1(q0,i0) actually needs lands
        # first at full DMA efficiency: mm1 starts ~12us instead of ~20us
        PKH = 4 * 128 * 512  # one [p, 4k, 512c] p-major half-piece
        xp = [
            xT[i * PKH : (i + 1) * PKH].rearrange("(p k c) -> p k c", p=128, k=4)
            for i in range(4)
        ]
        xi2 = xT[4 * PKH :].rearrange("(p k c) -> p k c", p=128, k=8)
        nc.gpsimd.dma_start(out=xt[:, 0:4, 0:512], in_=xp[0])
        nc.sync.dma_start(out=xt[:, 4:8, 0:512], in_=xp[1])

        # ================= phase 1: q/k projections + RoPE =================
        with ExitStack() as p1:
            p_t1 = p1.enter_context(tc.tile_pool(name="t1", bufs=1))
            p_w = p1.enter_context(tc.tile_pool(name="w", bufs=2))
            p_tmp = p1.enter_context(tc.tile_pool(name="tmp", bufs=4))
            p_sw = p1.enter_context(tc.tile_pool(name="sw", bufs=4))
            p_ps1 = p1.enter_context(tc.tile_pool(name="ps1", bufs=6, space="PSUM"))

            cos_sb = p_t1.tile([128, S], bf, tag="cos")
            s_sb = p_t1.tile([128, S], bf, tag="sin")
            wq_t = p_w.tile([128, 8, 8, 128], bf, tag="w", name="wq")
            wk_t = p_w.tile([128, 8, 8, 128], bf, tag="w", name="wk")
            # The 3 hw queues share ~300 GB/s aggregate: anything streaming
            # early steals bandwidth from the first-needed pieces. So: only
            # xt + the first wq chunks move early; wk/wv/tables ride behind
            # wq on scalar as FEW large descriptors (each dma issue costs
            # ~0.65us on the issuing engine's queue).
            PKW = 8 * 128 * 128  # one chunk's [p, k, 128] weight piece
            def wq_piece(c0, c1):
                return Wq[c0 * PKW : c1 * PKW].rearrange(
                    "(c p k m) -> p c k m", p=128, k=8, m=128
                )
            nc.scalar.dma_start(out=wq_t[:, 0:1], in_=wq_piece(0, 1))
            nc.gpsimd.dma_start(out=xt[:, 0:4, 512:1024], in_=xp[2])
            nc.sync.dma_start(out=xt[:, 4:8, 512:1024], in_=xp[3])
            nc.scalar.dma_start(out=wq_t[:, 1:2], in_=wq_piece(1, 2))
            nc.scalar.dma_start(out=wq_t[:, 2:5], in_=wq_piece(2, 5))
            nc.scalar.dma_start(out=wq_t[:, 5:8], in_=wq_piece(5, 8))
            nc.gpsimd.dma_start(out=xt[:, :, 1024:1040], in_=xi2)
            nc.gpsimd.dma_start(out=cos_sb, in_=COS2[:, :])
            nc.gpsimd.dma_start(out=s_sb, in_=S2[:, :])
            nc.gpsimd.dma_start(out=sel2, in_=SEL2[:, :])
            nc.scalar.dma_start(
                out=wk_t[:, :],
                in_=Wk[:].rearrange("(c p k m) -> p c k m", p=128, k=8, m=128),
            )
            nc.scalar.dma_start(
                out=wv_t[:, :],
                in_=Wv[:].rearrange("(c p k m) -> p c k m", p=128, k=8, m=512),
            )

            # PE warmup: scratch matmuls bring the clock up while DMAs land
            wa = p_tmp.tile([128, 512], bf, tag="wa", bufs=1)
            wb = p_tmp.tile([128, 128], bf, tag="wb", bufs=1)
            nc.vector.memset(wa, 0.0)
            nc.vector.memset(wb, 0.0)
            wps = p_ps1.tile([128, 512], f32, tag="mm1", name="warm_ps")
            for _w in range(12):
                nc.tensor.matmul(wps, wb, wa, start=True, stop=True)

            # memsets on the (otherwise idle) DVE; gpsimd only issues DMAs
            for c in range(NCHUNK):
                # all krz zeroing on gpsimd: ~20us of up-front DVE memsets
                # otherwise sit ahead of the first ropes in the DVE FIFO
                # (DVE is ~90% busy through phase 1)
                nc.gpsimd.memset(krz[c][0][64:128, :], 0.0)
                nc.gpsimd.memset(krz[c][1][0:64, :], 0.0)
                nc.gpsimd.memset(krz[c][0][0:64, 1040:1056], 0.0)
                nc.gpsimd.memset(krz[c][1][64:128, 1040:1056], 0.0)
            for j in range(NJT):
                r = rows_of(j)
                vx3 = vx[j][:, :1040].rearrange("p (h d) -> p h d", d=65)
                nc.gpsimd.memset(vx[j][:, 1040:1104], 0.0)
                nc.gpsimd.memset(vx3[:r, :, 64:65], 1.0)
                if j == 8:
                    # v8's 16 rows are replicated at partition offsets 32
                    # and 64 so the att@v j8 stationary can match the
                    # packed gpt moving tile's partition base
                    nc.gpsimd.memset(vx3[32 : 32 + r, :, 64:65], 1.0)
                    nc.gpsimd.memset(vx3[64 : 64 + r, :, 64:65], 1.0)
            # preload the exp ACT table so phase 2 doesn't pay the switch
            nc.scalar.activation(wa[0:1, 0:8], wa[0:1, 0:8], Exp, scale=0.0)

            def emit_mm1(which, w_t, c):
                # i-outer so the first psum chunk only needs xt cols 0:512
                raw = p_tmp.tile([128, S], bf, tag="raw", name=f"raw_{which}{c}")
                for i, (off, wdt) in enumerate(ICH3):
                    pss = p_ps1.tile(
                        [128, 512], f32, tag="mm1", name=f"mm1_{which}{c}_{i}"
                    )
                    for k in range(8):
                        nc.tensor.matmul(
                            pss[:, :wdt],
                            w_t[:, c, k : k + 1, :],
                            xt[:, k : k + 1, off : off + wdt],
                            start=(k == 0),
                            stop=(k == 7),
                        )
                    nc.scalar.copy(raw[:, off : off + wdt], pss[:, :wdt])
                return raw

            def emit_swap(which, c, raw):
                # rotate-half partner is partition p ^ 32 (hd layout
                # [r1,c1,r2,c2] per head): 4 contiguous-range SBUF->SBUF DMAs
                # on the otherwise-idle sync/gpsimd queues
                sw = p_sw.tile([128, S], bf, tag="sw", name=f"sw_{which}{c}")
                nc.sync.dma_start(out=sw[0:32, :], in_=raw[32:64, :])
                nc.sync.dma_start(out=sw[32:64, :], in_=raw[0:32, :])
                nc.sync.dma_start(out=sw[64:96, :], in_=raw[96:128, :])
                nc.sync.dma_start(out=sw[96:128, :], in_=raw[64:96, :])
                return sw

            def emit_rope(which, c, raw, sw):
                for off, wdt in ICH3:
                    t2 = p_tmp.tile([128, 512], bf, tag="t2")
                    nc.vector.tensor_mul(
                        t2[:, :wdt], sw[:, off : off + wdt], s_sb[:, off : off + wdt]
                    )
                    t1 = p_tmp.tile([128, 512], bf, tag="t1")
                    nc.vector.tensor_mul(
                        t1[:, :wdt], raw[:, off : off + wdt],
                        cos_sb[:, off : off + wdt],
                    )
                    if which == "q":
                        nc.vector.tensor_add(
                            qr[c][:, off : off + wdt], t1[:, :wdt], t2[:, :wdt]
                        )
                    else:
                        nc.vector.tensor_add(
                            krz[c][0][0:64, off : off + wdt],
                            t1[0:64, :wdt], t2[0:64, :wdt],
                        )
                        nc.vector.tensor_add(
                            krz[c][1][64:128, off : off + wdt],
                            t1[64:128, :wdt], t2[64:128, :wdt],
                        )

            def emit_v_tile_p1(j):
                # v tiles 0-2 on phase-1 PSUM, woven mid-stream (NOT at the
                # end: the phase-1->2 psum pool turnover barrier waits on the
                # last phase-1 psum consumer, and a trailing v-copy there
                # costs ~2us of PE idle at the boundary)
                r = rows_of(j)
                vx3 = vx[j][:, :1040].rearrange("p (h d) -> p h d", d=65)
                for ci in range(2):
                    psv = p_ps1.tile(
                        [128, 512], f32, tag="mm1", name=f"pv1_{j}_{ci}"
                    )
                    for k in range(8):
                        nc.tensor.matmul(
                            psv[:r, :],
                            xt[:, k : k + 1, j * 128 : j * 128 + r],
                            wv_t[:, ci, k : k + 1, :],
                            start=(k == 0),
                            stop=(k == 7),
                        )
                    nc.vector.tensor_copy(
                        vx3[:r, ci * 8 : (ci + 1) * 8, 0:64],
                        psv[:r, :].rearrange("p (h d) -> p h d", h=8),
                    )

            steps = [("q", c) for c in range(NCHUNK)] + [
                ("k", c) for c in range(NCHUNK)
            ]
            pending = None
            for si, (which, c) in enumerate(steps):
                raw = emit_mm1(which, wq_t if which == "q" else wk_t, c)
                sw = emit_swap(which, c, raw)
                if pending is not None:
                    emit_rope(*pending)
                pending = (which, c, raw, sw)
                if si in (11, 12, 13):  # k3/k4/k5 windows: wv has landed
                    emit_v_tile_p1(si - 11)

            emit_rope(*pending)

        # ============ phase 2: v-proj + attention (ACT-exp paced) ==========
        with ExitStack() as p2:
            p_wo = p2.enter_context(tc.tile_pool(name="wo", bufs=1))
            p_pt = p2.enter_context(tc.tile_pool(name="pt", bufs=NPT))
            p_cg = p2.enter_context(tc.tile_pool(name="cg", bufs=2))
            p_cs = p2.enter_context(tc.tile_pool(name="cs", bufs=2))
            p_rc = p2.enter_context(tc.tile_pool(name="rc", bufs=2))
            p_y = p2.enter_context(tc.tile_pool(name="y", bufs=4))
            p_st = p2.enter_context(tc.tile_pool(name="st", bufs=2, space="PSUM"))
            p_tl = p2.enter_context(tc.tile_pool(name="tl", bufs=1, space="PSUM"))
            p_po = p2.enter_context(tc.tile_pool(name="po", bufs=3, space="PSUM"))

            wo_t = p_wo.tile([128, 8, D], bf, tag="wo")
            Wo3 = Wo.rearrange("(k p) m -> p k m", p=128)
            nc.sync.dma_start(out=wo_t[:, 0:4, :], in_=Wo3[:, 0:4, :])
            nc.sync.dma_start(out=wo_t[:, 4:8, :], in_=Wo3[:, 4:8, :])

            pt = [
                p_pt.tile([128, (8 - NJD) * S], bf, tag="pt", name=f"pt{i}")
                for i in range(NPT)
            ]
            # j in {6,7} exp'd by DVE as bf16-bit-pattern i16s; att@v reads
            # these via .bitcast(bf16)
            p_pti = p2.enter_context(tc.tile_pool(name="pti", bufs=NPT))
            pti = [
                p_pti.tile([128, NJD * S], i16, tag="pti", name=f"pti{i}")
                for i in range(NPT)
            ]
            # ...except head 15, whose DVE-set js go on ACT (free at the
            # tail) writing real bf16 through a bitcast view of pti[0]
            # (h15's slot, never DVE-written): the DVE queue is the drain's
            # critical path, gating att@v(15) -> norm(7) -> final out-proj
            pex = pti[15 % NPT].bitcast(bf)
            # j8 (the 16-key tail tile) is handled per 4-head group: the 4
            # heads' [16,1040] score strips sit at partition offsets 0/32/
            # 64/96 of one shared tile so ONE exp covers all of them
            # (per-head j8 exps cost a full 1095ns for 16 rows each).
            p_gpt = p2.enter_context(tc.tile_pool(name="gpt", bufs=2))
            gpt = [
                p_gpt.tile([128, S], bf, tag="gpt", name=f"gpt{g}")
                for g in range(6)
            ]
            cs = [None] * NCHUNK  # per-chunk [2,S] denominator tiles
            # tl: [0:128) tails j0-7 (even head), [128:256) odd head,
            # [256:352) the 6 groups' j8 query-tails
            tl = p_tl.tile([128, 352], f32, tag="tl")
            nc.vector.memset(tl[:, 256:352], 0.0)

            def emit_v_tile(j):
                r = rows_of(j)
                vx3 = vx[j][:, :1040].rearrange("p (h d) -> p h d", d=65)
                for ci in range(2):
                    psv = p_po.tile([128, 512], f32, tag="po", name=f"pv{j}_{ci}")
                    for k in range(8):
                        nc.tensor.matmul(
                            psv[:r, :],
                            xt[:, k : k + 1, j * 128 : j * 128 + r],
                            wv_t[:, ci, k : k + 1, :],
                            start=(k == 0),
                            stop=(k == 7),
                        )
                    bases = (0, 32, 64) if j == 8 else (0,)
                    for bs in bases:
                        nc.vector.tensor_copy(
                            vx3[bs : bs + r, ci * 8 : (ci + 1) * 8, 0:64],
                            psv[:r, :].rearrange("p (h d) -> p h d", h=8),
                        )

            def emit_scores_j(h, j):
                c, hb = divmod(h, 2)
                ptf = pt[h % NPT]
                tb = (h % 2) * 128
                st = p_st.tile([128, 1024], f32, tag="st", name=f"st{h}_{j}")
                for off, wdt in ICH2:
                    nc.tensor.matmul(
                        st[:, off : off + wdt],
                        krz[c][hb][:, j * 128 : (j + 1) * 128],
                        qr[c][:, off : off + wdt],
                        start=True,
                        stop=True,
                    )
                nc.tensor.matmul(
                    tl[:, tb + j * 16 : tb + (j + 1) * 16],
                    krz[c][hb][:, j * 128 : (j + 1) * 128],
                    qr[c][:, 1024:1040],
                    start=True,
                    stop=True,
                )
                if JSP <= j < JSP + NJD and h == 15:
                    nc.scalar.activation(
                        pex[:, (j - JSP) * S : (j - JSP) * S + 1024], st[:, :],
                        Exp, scale=1.0 / np.sqrt(HD),
                    )
                elif JSP <= j < JSP + NJD:
                    # one-op DVE Schraudolph exp -> bf16 bits in the i16 tile
                    nc.vector.tensor_scalar(
                        pti[h % NPT][:, (j - JSP) * S : (j - JSP) * S + 1024],
                        st[:, :], SCHR_A, SCHR_B,
                        AluOpType.mult, AluOpType.add,
                    )
                else:
                    nc.scalar.activation(
                        ptf[:, ptreg(j) * S : ptreg(j) * S + 1024], st[:, :],
                        Exp, scale=1.0 / np.sqrt(HD),
                    )

            def emit_tails_exp_act(h):
                # needs only j0..JSP-1 tail columns of tl -> emit right
                # after scores_j(h, JSP-1): ACT finishes this head's stream
                # earlier, unblocking the next head's exps (st-slot chain)
                ptf = pt[h % NPT]
                ptv = ptf.rearrange("p (j q) -> p j q", q=S)
                tb = (h % 2) * 128
                nc.scalar.activation(
                    ptv[:, 0:JSP, 1024:1040],
                    tl[:, tb : tb + 16 * JSP].rearrange(
                        "p (j t) -> p j t", t=16
                    ),
                    Exp,
                    scale=1.0 / np.sqrt(HD),
                )

            def emit_tails_exp(h):
                ptf = pt[h % NPT]
                ptv = ptf.rearrange("p (j q) -> p j q", q=S)
                tb = (h % 2) * 128
                if JSP + NJD < 8:
                    nc.scalar.activation(
                        ptv[:, JSP : 8 - NJD, 1024:1040],
                        tl[:, tb + 16 * (JSP + NJD) : tb + 128].rearrange(
                            "p (j t) -> p j t", t=16
                        ),
                        Exp,
                        scale=1.0 / np.sqrt(HD),
                    )
                if h == 15:
                    nc.scalar.activation(
                        pex.rearrange("p (j q) -> p j q", q=S)[:, 0:NJD, 1024:1040],
                        tl[:, tb + 16 * JSP : tb + 16 * (JSP + NJD)].rearrange(
                            "p (j t) -> p j t", t=16
                        ),
                        Exp,
                        scale=1.0 / np.sqrt(HD),
                    )
                else:
                    ptiv = pti[h % NPT].rearrange("p (j q) -> p j q", q=S)
                    nc.vector.tensor_scalar(
                        ptiv[:, 0:NJD, 1024:1040],
                        tl[:, tb + 16 * JSP : tb + 16 * (JSP + NJD)].rearrange(
                            "p (j t) -> p j t", t=16
                        ),
                        SCHR_A, SCHR_B,
                        AluOpType.mult, AluOpType.add,
                    )

            def emit_group_j8(g):
                # scores + exp for the j8 key tile of heads 3g..3g+2, packed
                # at partition offsets 32m (AP base must be 0/32/64); the
                # 32-wide stationary (16 real + 16 zero key columns) writes
                # full 32-tall strips, and rows 96:128 are never read, so
                # the packed tile needs no memset
                stg = p_st.tile([128, 1024], f32, tag="st", name=f"stg{g}")
                for m in range(3):
                    hh = 3 * g + m
                    if hh >= H:
                        break
                    c, hb = divmod(hh, 2)
                    for off, wdt in ICH2:
                        nc.tensor.matmul(
                            stg[32 * m : 32 * m + 32, off : off + wdt],
                            krz[c][hb][:, 1024:1056],
                            qr[c][:, off : off + wdt],
                            start=True,
                            stop=True,
                        )
                    nc.tensor.matmul(
                        tl[32 * m : 32 * m + 32, 256 + g * 16 : 272 + g * 16],
                        krz[c][hb][:, 1024:1056],
                        qr[c][:, 1024:1040],
                        start=True,
                        stop=True,
                    )
                nh = min(3, H - 3 * g)
                nc.scalar.activation(
                    gpt[g][0 : 32 * nh, 0:1024],
                    stg[0 : 32 * nh, :],
                    Exp,
                    scale=1.0 / np.sqrt(HD),
                )
                nc.scalar.activation(
                    gpt[g][0 : 32 * nh, 1024:1040],
                    tl[0 : 32 * nh, 256 + g * 16 : 272 + g * 16],
                    Exp,
                    scale=1.0 / np.sqrt(HD),
                )

            cstage = {}
            otd = {}

            def emit_attv_part1(h, i):
                # first half of an att@v chunk's key accumulation; split so
                # the PE block between two scores matmuls stays short enough
                # that the ACT exp stream never drains its lookahead
                c, hb = divmod(h, 2)
                off, wdt = ICH3[i]
                ptf = pt[h % NPT]
                if otc[c] is None:
                    otc[c] = p_qk.tile([128, S], bf, tag="qk", name=f"otc{c}")
                if i == 0:
                    cstage[h] = p_cg.tile([65, S], f32, tag="cg", name=f"cst{h}")
                ot = p_po.tile([128, 512], f32, tag="po", name=f"ot{h}_{i}")
                otd[(h, i)] = ot
                for j in range(4):
                    nc.tensor.matmul(
                        ot[:, :wdt],
                        vx[j][:, h * 65 : h * 65 + 128],
                        ptf[:, j * S + off : j * S + off + wdt],
                        start=(j == 0),
                        stop=False,
                    )

            def emit_attv_part2(h, i):
                c, hb = divmod(h, 2)
                off, wdt = ICH3[i]
                ptf = pt[h % NPT]
                ptib = pex if h == 15 else pti[h % NPT].bitcast(bf)
                for j in range(4, 8):
                    rhs = (
                        ptf[:, ptreg(j) * S + off : ptreg(j) * S + off + wdt]
                        if not (JSP <= j < JSP + NJD)
                        else ptib[
                            :, (j - JSP) * S + off : (j - JSP) * S + off + wdt
                        ]
                    )
                    nc.tensor.matmul(
                        otd[(h, i)][:, :wdt],
                        vx[j][:, h * 65 : h * 65 + 128],
                        rhs,
                        start=False,
                        stop=False,
                    )
                ot = otd.pop((h, i))
                m = h % 3
                nc.tensor.matmul(
                    ot[:, :wdt],
                    vx[8][32 * m : 32 * m + TAIL, h * 65 : h * 65 + 128],
                    gpt[h // 3][32 * m : 32 * m + TAIL, off : off + wdt],
                    start=False,
                    stop=True,
                )
                # NOTE: moving these evacuations to ACT was tried and costs
                # +17us: the next head's exps queue behind them in ACT's
                # FIFO while they wait on the att@v matmuls (head-of-line).
                nc.vector.tensor_copy(
                    otc[c][hb * 64 : hb * 64 + 64, off : off + wdt],
                    ot[0:64, :wdt],
                )
                nc.vector.tensor_copy(
                    cstage[h][64:65, off : off + wdt], ot[64:65, :wdt]
                )
                if i == 2:
                    if cs[c] is None:
                        cs[c] = p_cs.tile([2, S], f32, tag="cs", name=f"cs{c}")
                    (nc.sync if h % 2 == 0 else nc.gpsimd).dma_start(
                        out=cs[c][hb : hb + 1, :], in_=cstage[h][64:65, :]
                    )

            def emit_attv_chunk(h, i):
                emit_attv_part1(h, i)
                emit_attv_part2(h, i)

            def emit_norm(c):
                rcp = p_rc.tile([2, S], f32, tag="rcp", name=f"rcp{c}")
                nc.vector.reciprocal_approx_fast(rcp, cs[c])
                rcpb = p_rc.tile([2, S], bf, tag="rcpb", name=f"rcpb{c}")
                nc.vector.tensor_copy(rcpb, rcp)
                for off, wdt in ICH3:
                    psb = p_po.tile([128, 512], f32, tag="po", name=f"nm{c}_{off}")
                    nc.tensor.matmul(
                        psb[:, :wdt], sel2, rcpb[:, off : off + wdt],
                        start=True, stop=True,
                    )
                    nc.vector.tensor_mul(
                        otc[c][:, off : off + wdt],
                        otc[c][:, off : off + wdt],
                        psb[:, :wdt],
                    )

            # phase 1.5: rest of V projection woven under heads 0-1
            vq = [3, 4, 5, 6, 7]
            for h in (0, 1):
                for j in range(8):
                    emit_scores_j(h, j)
                    if j == JSP - 1:
                        emit_tails_exp_act(h)
                    if j in (1, 3, 5) and vq:
                        emit_v_tile(vq.pop(0))
                emit_tails_exp(h)
                if h % 3 == 0:
                    emit_group_j8(h // 3)
            emit_v_tile(8)

            # steady state: head h scores woven with att@v of head h-2;
            # h15 additionally absorbs att@v(14) (its exps are done by then)
            # att@v chunks are emitted in two halves around the next scores
            # matmul so the PE block between score pairs stays short and the
            # ACT exp stream keeps its lookahead fed
            for h in range(2, H):
                for j in range(8):
                    emit_scores_j(h, j)
                    if j == JSP - 1:
                        emit_tails_exp_act(h)
                    if h < 15:
                        if j in (0, 2, 4):
                            emit_attv_part1(h - 2, {0: 0, 2: 1, 4: 2}[j])
                        if j in (1, 3, 5):
                            emit_attv_part2(h - 2, {1: 0, 3: 1, 5: 2}[j])
                    else:
                        if j in (0, 2, 4):
                            emit_attv_chunk(h - 2, {0: 0, 2: 1, 4: 2}[j])
                        if j in (1, 3, 5):
                            emit_attv_chunk(14, {1: 0, 3: 1, 5: 2}[j])
                emit_tails_exp(h)
                # group 5 (head 15's j8) hoisted to h14's window: its ACT
                # exp otherwise sits right on the drain gate (att@v(15) ->
                # norm(7) -> final out-proj); krz[7] is long since ready
                if h % 3 == 0 and h < 15:
                    emit_group_j8(h // 3)
                if h == 14:
                    emit_group_j8(5)
                # norm(c) needs att@v(2c+1), complete at the end of head
                # 2c+3's window -> emit at h = 2c+4; norm(6) right after
                # att@v(13) lands inside h15
                if h >= 4 and h % 2 == 0:
                    emit_norm((h - 4) // 2)
                if h == 15:
                    emit_norm(6)

            # drain: att@v for head 15, then norm(7)
            for i in range(3):
                emit_attv_chunk(15, i)

            def emit_yproj_mm(it, cs_, start, stop):
                r = rows_of(it)
                if it not in yps:
                    yps[it] = p_st.tile([128, 1024], f32, tag="st", name=f"y{it}")
                for ci in range(2):
                    for c in cs_:
                        nc.tensor.matmul(
                            yps[it][:r, ci * 512 : (ci + 1) * 512],
                            otc[c][:, it * 128 : it * 128 + r],
                            wo_t[:, c : c + 1, ci * 512 : (ci + 1) * 512],
                            start=(start and c == cs_[0]),
                            stop=(stop and c == cs_[-1]),
                        )

            def emit_yproj_out(it):
                # output DMAs only on sync/gpsimd: a nc.scalar.dma_start
                # issue (~0.7us) between ysb copies in the ACT FIFO slows the
                # po/st slot recycle and starves the PE at the drain
                r = rows_of(it)
                for ci in range(2):
                    ysb = p_y.tile([128, 512], bf, tag="ysb")
                    eng = (nc.scalar.copy, nc.vector.tensor_copy)[ci]
                    eng(ysb[:r, :], yps[it][:r, ci * 512 : (ci + 1) * 512])
                    qs = (nc.sync, nc.gpsimd)[ci]
                    qs.dma_start(
                        out=OUT[it * 128 : it * 128 + r, ci * 512 : (ci + 1) * 512],
                        in_=ysb[:r, :],
                    )

            # output projection: it0/it1 accumulate chunks 0-6 BEFORE
            # norm(7) is emitted, hiding the chunk-7 reciprocal chain
            # (cstage DMA -> rcp -> selector matmul -> DVE mul) behind PE
            # work; chunk 7 joins as the final accumulation step.
            # yps tiles are created lazily so the st-slot ring only
            # contains tiles actually used (it2 runs on po slots; an unused
            # yps[2] would misalign the ring and stall it3 behind it1)
            yps = {}
            emit_yproj_mm(0, list(range(7)), True, False)
            emit_yproj_mm(1, list(range(7)), True, False)
            # it2's partial runs on po slots (both st slots are held open by
            # it0/it1) to keep the PE fed through norm(7)'s reciprocal chain
            yp2 = [
                p_po.tile([128, 512], f32, tag="po", name=f"yp2_{ci}")
                for ci in range(2)
            ]
            for ci in range(2):
                for c in range(7):
                    nc.tensor.matmul(
                        yp2[ci][:, :],
                        otc[c][:, 2 * 128 : 3 * 128],
                        wo_t[:, c : c + 1, ci * 512 : (ci + 1) * 512],
                        start=(c == 0),
                        stop=False,
                    )
            emit_norm(7)
            emit_yproj_mm(0, [7], False, True)
            emit_yproj_out(0)
            emit_yproj_mm(1, [7], False, True)
            emit_yproj_out(1)
            for ci in range(2):
                nc.tensor.matmul(
                    yp2[ci][:, :],
                    otc[7][:, 2 * 128 : 3 * 128],
                    wo_t[:, 7:8, ci * 512 : (ci + 1) * 512],
                    start=False,
                    stop=True,
                )
                ysb = p_y.tile([128, 512], bf, tag="ysb")
                (nc.scalar.copy, nc.vector.tensor_copy)[ci](ysb, yp2[ci][:, :])
                (nc.sync, nc.gpsimd)[ci].dma_start(
                    out=OUT[2 * 128 : 3 * 128, ci * 512 : (ci + 1) * 512],
                    in_=ysb,
                )
            # it3-8 on per-ci [128,512] po tiles (3-slot ring) instead of the
            # 2-slot [128,1024] st ring: the ysb-copy latency of block N no
            # longer exposes itself as a PE stall before block N+2
            for it in range(3, NJT):
                r = rows_of(it)
                for ci in range(2):
                    ypc = p_po.tile([128, 512], f32, tag="po", name=f"yf{it}_{ci}")
                    for c in range(8):
                        nc.tensor.matmul(
                            ypc[:r, :],
                            otc[c][:, it * 128 : it * 128 + r],
                            wo_t[:, c : c + 1, ci * 512 : (ci + 1) * 512],
                            start=(c == 0),
                            stop=(c == 7),
                        )
                    ysb = p_y.tile([128, 512], bf, tag="ysb")
                    eng = (nc.scalar.copy, nc.vector.tensor_copy)[(it + ci) % 2]
                    eng(ysb[:r, :], ypc[:r, :])
                    qs = (nc.sync, nc.gpsimd)[ci]
                    qs.dma_start(
                        out=OUT[it * 128 : it * 128 + r, ci * 512 : (ci + 1) * 512],
                        in_=ysb[:r, :],
                    )


def _build():
    global _compiled
    if _compiled is not None:
        return _compiled
    import concourse.bass as bass  # noqa: F401
    import concourse.mybir as mybir
    import concourse.tile as tile
    from concourse import bacc

    nc = bacc.Bacc("TRN2", target_bir_lowering=False, debug=False)
    bf = mybir.dt.bfloat16
    aps = {
        "xT": nc.dram_tensor("xT", [D * S], bf, kind="ExternalInput").ap(),
        "Wq": nc.dram_tensor("Wq", [D * H * HD], bf, kind="ExternalInput").ap(),
        "Wk": nc.dram_tensor("Wk", [D * H * HD], bf, kind="ExternalInput").ap(),
        "Wv": nc.dram_tensor("Wv", [D * H * HD], bf, kind="ExternalInput").ap(),
        "Wo": nc.dram_tensor("Wo", [H * HD, D], bf, kind="ExternalInput").ap(),
        "COS2": nc.dram_tensor("COS2", [128, S], bf, kind="ExternalInput").ap(),
        "S2": nc.dram_tensor("S2", [128, S], bf, kind="ExternalInput").ap(),
        "SEL2": nc.dram_tensor("SEL2", [2, 128], bf, kind="ExternalInput").ap(),
        "out": nc.dram_tensor("out", [S, D], bf, kind="ExternalOutput").ap(),
    }
    with tile.TileContext(nc) as tc:
        _build_body(nc, tc, tile, mybir, aps)
    nc.compile()
    _compiled = nc
    return nc


def _install_trace_shim():
    """The agent image's antenv lacks axon_hooks, so run_bass_kernel_spmd's
    trace path can't find the NTFF profile hook trn_boot would have set.
    Recreate the module and install the ctypes hook; skip the S3 artifact
    upload (no creds needed for local timing)."""
    import sys
    import types

    if "antenv.axon_hooks" not in sys.modules:
        import antenv  # noqa: F401

        mod = types.ModuleType("antenv.axon_hooks")
        mod._hook = None

        def set_axon_ntff_profile_hook(h):
            mod._hook = h

        def get_axon_ntff_profile_hook():
            return mod._hook

        mod.set_axon_ntff_profile_hook = set_axon_ntff_profile_hook
        mod.get_axon_ntff_profile_hook = get_axon_ntff_profile_hook
        sys.modules["antenv.axon_hooks"] = mod

    import antenv.axon_hooks as ah

    if ah.get_axon_ntff_profile_hook() is None:
        from trn_agent_boot.trn_boot import _ntff_profile_via_ctypes

        ah.set_axon_ntff_profile_hook(
            _ntff_profile_via_ctypes("/opt/axon/libaxon_pjrt.so")
        )

    import concourse.bass_utils as bu

    bu.upload_artifacts = lambda tmpdir: f"local://{tmpdir}"


def run(inputs, trace=False):
    """Returns (output (8,1040,1024) f32, exec_time_ns or None)."""
    if trace:
        _install_trace_shim()
    from concourse.bass_utils import run_bass_kernel_spmd

    nc = _build()
    x = np.asarray(inputs["x"], np.float32)
    # q/k weights: permute hd components to the [r1,c1,r2,c2] swap-by-p^32
    # layout (must match _rope_tables and the DMA swap in the kernel)
    wq = np.asarray(inputs["Wq"], np.float32)[:, :, HD_PERM]
    wq = wq.reshape(D, H * HD).astype(BF16)
    wk = np.asarray(inputs["Wk"], np.float32)[:, :, HD_PERM]
    wk = wk.reshape(D, H * HD).astype(BF16)
    wv = np.asarray(inputs["Wv"], np.float32).reshape(D, H * HD).astype(BF16)
    wo = np.asarray(inputs["Wo"], np.float32).reshape(H * HD, D).astype(BF16)

    # piece-major dram packing (see the kernel's input-DMA comment):
    # weights by 128-col chunk [c][k,p,m], wv by 512-col half, xt by ICH
    # column block [i][k,p,cols] - the first-needed pieces stream first
    def pack_w(w, m):
        # [c][p][k][m]: per (chunk, partition) runs are contiguous on BOTH
        # the dram and sbuf side (sbuf weight tiles are chunk-major)
        return np.ascontiguousarray(
            w.reshape(8, 128, w.shape[1] // m, m).transpose(2, 1, 0, 3)
        ).ravel()

    wq = pack_w(wq, 128)
    wk = pack_w(wk, 128)
    wv = pack_w(wv, 512)
    cos2, s2 = _rope_tables()
    sel2b = np.zeros((2, 128), np.float32)
    sel2b[0, 0:64] = 1.0
    sel2b[1, 64:128] = 1.0
    shared = {
        "Wq": wq, "Wk": wk, "Wv": wv, "Wo": wo,
        "COS2": cos2, "S2": s2, "SEL2": sel2b.astype(BF16),
    }
    def pack_x(xb):
        xk = np.ascontiguousarray(xb.T).astype(BF16).reshape(8, 128, S)
        pieces = [
            xk[0:4, :, 0:512], xk[4:8, :, 0:512],
            xk[0:4, :, 512:1024], xk[4:8, :, 512:1024],
            xk[:, :, 1024:1040],
        ]
        return np.concatenate(
            [np.ascontiguousarray(p.transpose(1, 0, 2)).ravel() for p in pieces]
        )

    in_maps = [dict(shared, xT=pack_x(x[b])) for b in range(B)]
    res = run_bass_kernel_spmd(nc, in_maps, core_ids=list(range(B)), trace=trace)
    out = np.stack([np.asarray(r["out"], np.float32) for r in res.results], axis=0)
    return out, res.exec_time_ns


def kernel(x, Wq, Wk, Wv, Wo):
    out, _ = run({"x": x, "Wq": Wq, "Wk": Wk, "Wv": Wv, "Wo": Wo})
    return out

